# revision 28
# baseline (speedup 1.0000x reference)
"""Trainium2 Bass kernel for nn_Caption (LSTM caption decoder).

Distribution: pure data-parallel over batch (128 -> 8 cores x 16), no
collectives. Per core: x0 projection GEMM, embedding gather (device),
input-gate GEMM, 40-step LSTM recurrence, vocab GEMM [640,512]@[512,10000].

Layout strategy: all GEMM operands bf16 (fp32 PSUM accumulation); weights
host-transposed so the contraction dim lands on partitions; outputs
produced in T-layout (feature on partitions) so biases fuse into ACT
copies as per-partition bias. LSTM runs B-layout (batch on partitions)
with per-step h transposed via PE into hiddensT, which is consumed
directly by the vocab GEMM. xg is injected into the gates PSUM via
identity matmuls (t-blocks padded to 32 partitions for alignment).
"""
import sys

sys.path.insert(0, "/opt/trn_rl_repo")

import numpy as np
import ml_dtypes

import concourse.bass as bass
import concourse.tile as tile
from concourse import bacc, mybir
from concourse.bass_utils import run_bass_kernel_spmd
from concourse.masks import make_identity

BF = mybir.dt.bfloat16
F32 = mybir.dt.float32
I32 = mybir.dt.int32
bfnp = ml_dtypes.bfloat16

B, F, E, H, V, T = 128, 1536, 512, 512, 10000, 40
NCORES = 8
BC = B // NCORES          # 16 batch rows per core
TB = 32                   # padded t-block width (partition alignment)
NTB = T * TB              # 1280 padded (t,b) columns
NB = T * BC               # 640 real (t,b) columns
G4 = 4 * H                # 2048 gate dims, order [i, f, o, g]
VP = 10240               # padded vocab (80 tiles of 128, 20 quads)
NVT = VP // 128           # 80 vocab tiles
NVQ = NVT // 4            # 20 vocab quads

_CACHE = {}


def _build():
    if "nc" in _CACHE:
        return _CACHE["nc"]
    nc = bacc.Bacc("TRN2", target_bir_lowering=False, debug=False,
                   num_devices=NCORES)

    featT_d = nc.dram_tensor("featT", [F, BC], BF, kind="ExternalInput")
    idx_d = nc.dram_tensor("idx", [NTB, 1], I32, kind="ExternalInput")
    emb_d = nc.dram_tensor("embt", [V, E], BF, kind="ExternalInput")
    WinT_d = nc.dram_tensor("WinT", [F, E], BF, kind="ExternalInput")
    WihT_d = nc.dram_tensor("WihT", [E, G4], BF, kind="ExternalInput")
    WhhT_d = nc.dram_tensor("WhhT", [H, G4], BF, kind="ExternalInput")
    bcomb_d = nc.dram_tensor("bcomb", [G4], F32, kind="ExternalInput")
    bin_d = nc.dram_tensor("bin", [E], F32, kind="ExternalInput")
    boutb_d = nc.dram_tensor("boutb", [VP], BF, kind="ExternalInput")
    WoutTt_d = nc.dram_tensor("WoutTt", [NVQ, 128, 4, 512], BF,
                              kind="ExternalInput")
    out_d = nc.dram_tensor("out_q", [3, NVQ * 4, 128, 256], F32,
                           kind="ExternalOutput")

    with tile.TileContext(nc) as tc:
        with (
            tc.tile_pool(name="consts", bufs=1) as consts,
            tc.tile_pool(name="big", bufs=1) as big,
            tc.tile_pool(name="state", bufs=2) as state,
            tc.tile_pool(name="work", bufs=3) as work,
            tc.tile_pool(name="wpool", bufs=4) as wpool,
            tc.tile_pool(name="lpool", bufs=3) as lpool,
        ):
            # ---- index load + constants ----
            idx_sb = consts.tile([128, 10, 1], I32)
            nc.gpsimd.dma_start(
                idx_sb[:], idx_d.ap().rearrange("(j p) o -> p j o", p=128))
            identf = consts.tile([128, 128], F32)
            make_identity(nc, identf[:])
            identb = consts.tile([128, 128], BF)
            nc.vector.tensor_copy(identb[:], identf[:])

            WihT_sb = big.tile([128, 4, G4], BF, tag="wih")
            nc.sync.dma_start(
                WihT_sb[:], WihT_d.ap().rearrange("(k p) n -> p k n", p=128))
            WinT_sb = big.tile([128, 12, E], BF, tag="win")
            nc.sync.dma_start(
                WinT_sb[:], WinT_d.ap().rearrange("(k p) n -> p k n", p=128))
            featT_sb = consts.tile([128, 12, BC], BF)
            nc.sync.dma_start(
                featT_sb[:], featT_d.ap().rearrange("(k p) b -> p k b", p=128))
            WhhT_sb = big.tile([128, 4, G4], BF, tag="whh")
            nc.sync.dma_start(
                WhhT_sb[:], WhhT_d.ap().rearrange("(k p) n -> p k n", p=128))
            bias_bc = big.tile([128, G4], F32, tag="biasbc")
            nc.sync.dma_start(
                bias_bc[:],
                bass.AP(tensor=bcomb_d, offset=0, ap=[[0, 128], [1, G4]]))
            bin_sb = consts.tile([128, 4], F32)
            nc.sync.dma_start(
                bin_sb[:], bin_d.ap().rearrange("(k p) -> p k", p=128))
            boutb_sb = consts.tile([1, VP], BF)
            nc.sync.dma_start(boutb_sb[:], boutb_d.ap()[None, :])
            ones_sb = consts.tile([1, 256], BF)
            nc.gpsimd.memset(ones_sb[:], 1.0)

            # ---- embedding gather -> seqT (transposed via PE) ----
            seqT = big.tile([128, 4, NTB], BF, tag="seqT")
            with tc.tile_pool(name="psA", bufs=3, space="PSUM") as psA:
                for j in range(10):
                    gt = work.tile([128, E], BF, tag="gather")
                    nc.gpsimd.indirect_dma_start(
                        out=gt[:], out_offset=None, in_=emb_d.ap(),
                        in_offset=bass.IndirectOffsetOnAxis(
                            ap=idx_sb[:, j, :], axis=0))
                    for e in range(4):
                        pst = psA.tile([128, 128], BF, space="PSUM", tag="tr")
                        nc.tensor.transpose(
                            pst[:], gt[:, e * 128:(e + 1) * 128], identb[:])
                        nc.scalar.copy(
                            seqT[:, e, j * 128:(j + 1) * 128], pst[:])

                # ---- x0T = W_inT.T @ featT + b_in -> seqT[:, :, 0:BC] ----
                for m in range(4):
                    ps = psA.tile([128, BC], F32, space="PSUM", tag="x0")
                    for k in range(12):
                        nc.tensor.matmul(
                            ps[:], lhsT=WinT_sb[:, k, m * 128:(m + 1) * 128],
                            rhs=featT_sb[:, k, :],
                            start=(k == 0), stop=(k == 11))
                    nc.scalar.activation(
                        seqT[:, m, 0:BC], ps[:],
                        mybir.ActivationFunctionType.Identity,
                        bias=bin_sb[:, m:m + 1])

            # ---- LSTM + interleaved xg / vocab GEMM ----
            VWIN = [(0, 16), (16, 32), (32, 40)]
            hid_w = [big.tile([128, 4, t1 - t0, BC], BF, tag=f"hid{w}",
                              name=f"hid{w}")
                     for w, (t0, t1) in enumerate(VWIN)]
            xg_tiles = [big.tile([128, G4], BF, tag=f"xg{mt}", name=f"xg{mt}")
                        for mt in range(10)]

            def hid_of(t):
                for w, (t0, t1) in enumerate(VWIN):
                    if t < t1:
                        return hid_w[w], t - t0
                raise AssertionError

            lstm_ps = tc.tile_pool(name="psGates", bufs=1, space="PSUM")
            htr_ps = tc.tile_pool(name="psHtr", bufs=2, space="PSUM")
            xg_ps = tc.tile_pool(name="psXg", bufs=2, space="PSUM")
            voc_ps = tc.tile_pool(name="psVoc", bufs=2, space="PSUM")
            gpsum = lstm_ps.__enter__()
            tpsum = htr_ps.__enter__()
            xgpsum = xg_ps.__enter__()
            vpsum = [None]

            def emit_xg_mtile(mt):
                for n in range(4):
                    ps = xgpsum.tile([128, 512], F32, space="PSUM", tag="xgps")
                    for k in range(4):
                        nc.tensor.matmul(
                            ps[:],
                            lhsT=seqT[:, k, mt * 128:(mt + 1) * 128],
                            rhs=WihT_sb[:, k, n * 512:(n + 1) * 512],
                            start=(k == 0), stop=(k == 3))
                    nc.vector.tensor_add(
                        xg_tiles[mt][:, n * 512:(n + 1) * 512], ps[:],
                        bias_bc[:, n * 512:(n + 1) * 512])

            vunits = []
            ncopy = [0]

            def emit_vocab_unit(vtq, w):
                t0, t1 = VWIN[w]
                nb = (t1 - t0) * BC
                wt = wpool.tile([128, 4, 512], BF, tag="wout")
                nc.sync.dma_start(wt[:], WoutTt_d.ap()[vtq])
                lsb = lpool.tile([128, 4, 256], F32, tag="lout")
                for pair in range(2):
                    vps = vpsum[0].tile([128, 512], F32, space="PSUM",
                                        tag="vps")
                    for half in range(2):
                        sub = pair * 2 + half
                        vt = vtq * 4 + sub
                        hsl = slice(half * 256, half * 256 + nb)
                        for k in range(4):
                            nc.tensor.matmul(
                                vps[:, hsl],
                                lhsT=wt[:, k, sub * 128:(sub + 1) * 128],
                                rhs=hid_w[w][:, k, :, :],
                                start=(k == 0 and half == 0), stop=False)
                        nc.tensor.matmul(
                            vps[:, hsl],
                            lhsT=boutb_sb[0:1, vt * 128:(vt + 1) * 128],
                            rhs=ones_sb[0:1, 0:nb],
                            start=False, stop=(half == 1))
                    dst = lsb[:, 2 * pair:2 * pair + 2, 0:256] \
                        .rearrange("p s c -> p (s c)")
                    if pair == 0:
                        nc.scalar.copy(dst, vps[:])
                    else:
                        nc.vector.tensor_copy(dst, vps[:])
                nc.gpsimd.dma_start(
                    out_d.ap()[w, vtq * 4:(vtq + 1) * 4, :, :]
                    .rearrange("s p c -> p s c"),
                    lsb[:])

            emit_xg_mtile(0)
            emit_xg_mtile(1)

            HH = H // 2
            c_prev = None
            for t in range(T):
                if t == 16:
                    xg_ps.__exit__(None, None, None)
                    vpsum[0] = voc_ps.__enter__()
                mt, po = (t * TB) // 128, (t * TB) % 128
                gchunk = {}
                for n in (3, 0, 1, 2):
                    gchunk[n] = gpsum.tile(
                        [BC, 512], F32, space="PSUM", tag=f"gates{n}",
                        name=f"gates{n}")
                sig_i = state.tile([BC, H], F32, tag="sigi")
                sig_f = state.tile([BC, H], F32, tag="sigf")
                sig_o = state.tile([BC, H], F32, tag="sigo")
                g_t = state.tile([BC, H], F32, tag="g")
                act_of = {3: (g_t, mybir.ActivationFunctionType.Tanh),
                          0: (sig_i, mybir.ActivationFunctionType.Sigmoid),
                          1: (sig_f, mybir.ActivationFunctionType.Sigmoid),
                          2: (sig_o, mybir.ActivationFunctionType.Sigmoid)}
                for n in (3, 0, 1, 2):
                    ns = slice(n * 512, (n + 1) * 512)
                    if t > 0:
                        hsrc, trel = hid_of(t - 1)
                        for k in range(4):
                            nc.tensor.matmul(
                                gchunk[n][:],
                                lhsT=hsrc[:, k, trel, :],
                                rhs=WhhT_sb[:, k, ns],
                                start=(k == 0), stop=False)
                    nc.tensor.matmul(
                        gchunk[n][:],
                        lhsT=identb[po:po + BC, po:po + BC],
                        rhs=xg_tiles[mt][po:po + BC, ns],
                        start=(t == 0), stop=True,
                        tile_position=(po, 0))
                    dst, fn = act_of[n]
                    if n == 2:
                        nc.scalar.activation(dst[:, 0:256], gchunk[n][:, 0:256], fn)
                        nc.scalar.activation(dst[:, 256:512], gchunk[n][:, 256:512], fn)
                    else:
                        nc.scalar.activation(dst[:], gchunk[n][:], fn)

                hdst, trel = hid_of(t)
                c_new = [None, None]
                for half in range(2):
                    hs = slice(half * HH, (half + 1) * HH)
                    ig = state.tile([BC, HH], F32, tag=f"ig{half}")
                    nc.vector.tensor_mul(ig[:], sig_i[:, hs], g_t[:, hs])
                    cn = state.tile([BC, HH], F32, tag=f"c{half}")
                    if t == 0:
                        nc.vector.tensor_copy(cn[:], ig[:])
                    else:
                        cf = state.tile([BC, HH], F32, tag=f"cf{half}")
                        nc.vector.tensor_mul(
                            cf[:], sig_f[:, hs], c_prev[half][:])
                        nc.vector.tensor_add(cn[:], cf[:], ig[:])
                    c_new[half] = cn
                    tc_t = state.tile([BC, HH], F32, tag=f"tanhc{half}")
                    nc.scalar.activation(
                        tc_t[:], cn[:], mybir.ActivationFunctionType.Tanh)
                    h_bf = state.tile([BC, HH], BF, tag=f"h{half}")
                    nc.vector.tensor_mul(h_bf[:], sig_o[:, hs], tc_t[:])
                    pst = tpsum.tile([128, 2 * BC], BF, space="PSUM",
                                     tag="htr")
                    for e in range(2):
                        nc.tensor.transpose(
                            pst[:, e * BC:(e + 1) * BC],
                            h_bf[:, e * 128:(e + 1) * 128],
                            identb[0:BC, 0:BC])
                    dstap = hdst[:, 2 * half:2 * half + 2, trel, :]
                    srcap = pst[:].rearrange("p (k b) -> p k b", k=2)
                    if half == 0:
                        nc.scalar.copy(dstap, srcap)
                    else:
                        nc.vector.tensor_copy(dstap, srcap)
                c_prev = c_new

                # interleaved filler work
                if t < 16 and t % 2 == 0 and t // 2 + 2 < 10:
                    emit_xg_mtile(t // 2 + 2)
                for w, (t0, t1) in enumerate(VWIN):
                    if t == t1 - 1:
                        vunits.extend((vtq, w) for vtq in range(NVQ))
                if t >= 16:
                    if vunits:
                        emit_vocab_unit(*vunits.pop(0))

            # vocab tail
            while vunits:
                emit_vocab_unit(*vunits.pop(0))

            voc_ps.__exit__(None, None, None)
            htr_ps.__exit__(None, None, None)
            lstm_ps.__exit__(None, None, None)

    nc.compile()
    _CACHE["nc"] = nc
    return nc


def kernel(features, seqs, lengths, W_in, b_in, emb, W_ih, W_hh, b_ih, b_hh,
           W_out, b_out):
    f32 = lambda x: np.asarray(x, dtype=np.float32)
    bf = lambda x: np.ascontiguousarray(f32(x)).astype(bfnp)
    features, seqs = f32(features), np.asarray(seqs).astype(np.int64)
    # gate order [i, f, o, g]
    perm = np.concatenate([np.arange(0, 2 * H), np.arange(3 * H, 4 * H),
                           np.arange(2 * H, 3 * H)])
    WinT = bf(f32(W_in).T)                     # [F, E]
    WihT = np.ascontiguousarray(bf(f32(W_ih).T)[:, perm])
    WhhT = np.ascontiguousarray(bf(f32(W_hh).T)[:, perm])
    bcomb = np.ascontiguousarray((f32(b_ih) + f32(b_hh))[perm])
    emb_b = bf(emb)
    WoutT = np.zeros((H, VP), dtype=bfnp)
    WoutT[:, :V] = bf(f32(W_out).T)
    # quad-tiled layout [vtq, p, k, v4]: element = WoutT[k*128+p, vtq*512+v4]
    WoutTt = np.ascontiguousarray(
        WoutT.reshape(4, 128, NVQ, 512).transpose(2, 1, 0, 3))
    boutp = np.zeros((VP,), dtype=bfnp)
    boutp[:V] = bf(b_out)
    binp = f32(b_in)

    nc = _build()
    in_maps = []
    for c in range(NCORES):
        bs = slice(c * BC, (c + 1) * BC)
        featT = bf(features[bs].T)             # [F, BC]
        idx = np.zeros((T, TB), np.int64)
        idx[1:, :BC] = seqs[bs].T              # t-major, t=0 block dummy
        in_maps.append({
            "featT": featT,
            "idx": idx.reshape(NTB, 1).astype(np.int32),
            "embt": emb_b,
            "WinT": WinT, "WihT": WihT, "WhhT": WhhT,
            "bcomb": bcomb, "bin": binp, "boutb": boutp,
            "WoutTt": WoutTt,
        })
    _CACHE["last_in_maps"] = in_maps
    res = run_bass_kernel_spmd(nc, in_maps, list(range(NCORES)))
    out = np.empty((B, T, V), np.float32)
    wlens = [256, 256, 128]
    for c in range(NCORES):
        oq = res.results[c]["out_q"]           # [3, 80, 128, 256]
        parts = [oq[w].reshape(VP, 256)[:V, :wlens[w]] for w in range(3)]
        lt = np.concatenate(parts, axis=1)     # [V, 640]
        out[c * BC:(c + 1) * BC] = (
            lt.reshape(V, T, BC).transpose(2, 1, 0))
    return out


# revision 29
# speedup vs baseline: 1.0433x; 1.0433x over previous
"""Trainium2 Bass kernel for nn_Caption (LSTM caption decoder).

Distribution: pure data-parallel over batch (128 -> 8 cores x 16), no
collectives. Per core: x0 projection GEMM, embedding gather (device),
input-gate GEMM, 40-step LSTM recurrence, vocab GEMM [640,512]@[512,10000].

Layout strategy: all GEMM operands bf16 (fp32 PSUM accumulation); weights
host-transposed so the contraction dim lands on partitions; outputs
produced in T-layout (feature on partitions) so biases fuse into ACT
copies as per-partition bias. LSTM runs B-layout (batch on partitions)
with per-step h transposed via PE into hiddensT, which is consumed
directly by the vocab GEMM. xg is injected into the gates PSUM via
identity matmuls (t-blocks padded to 32 partitions for alignment).
"""
import sys

sys.path.insert(0, "/opt/trn_rl_repo")

import numpy as np
import ml_dtypes

import concourse.bass as bass
import concourse.tile as tile
from concourse import bacc, mybir
from concourse.bass_utils import run_bass_kernel_spmd
from concourse.masks import make_identity

BF = mybir.dt.bfloat16
F32 = mybir.dt.float32
I32 = mybir.dt.int32
bfnp = ml_dtypes.bfloat16

B, F, E, H, V, T = 128, 1536, 512, 512, 10000, 40
NCORES = 8
BC = B // NCORES          # 16 batch rows per core
TB = 32                   # padded t-block width (partition alignment)
NTB = T * TB              # 1280 padded (t,b) columns
NB = T * BC               # 640 real (t,b) columns
G4 = 4 * H                # 2048 gate dims, order [i, f, o, g]
VP = 10240               # padded vocab (80 tiles of 128, 20 quads)
NVT = VP // 128           # 80 vocab tiles
NVQ = NVT // 4            # 20 vocab quads

_CACHE = {}


def _build():
    if "nc" in _CACHE:
        return _CACHE["nc"]
    nc = bacc.Bacc("TRN2", target_bir_lowering=False, debug=False,
                   num_devices=NCORES)

    featT_d = nc.dram_tensor("featT", [F, BC], BF, kind="ExternalInput")
    idx_d = nc.dram_tensor("idx", [NTB, 1], I32, kind="ExternalInput")
    emb_d = nc.dram_tensor("embt", [V, E], BF, kind="ExternalInput")
    WinT_d = nc.dram_tensor("WinT", [128, 12, E], BF, kind="ExternalInput")
    WihT_d = nc.dram_tensor("WihT", [128, 4, G4], BF, kind="ExternalInput")
    WhhT_d = nc.dram_tensor("WhhT", [128, 4, G4], BF, kind="ExternalInput")
    bcomb_d = nc.dram_tensor("bcomb", [G4], F32, kind="ExternalInput")
    bin_d = nc.dram_tensor("bin", [E], F32, kind="ExternalInput")
    ident_d = nc.dram_tensor("ident", [128, 128], BF, kind="ExternalInput")
    WoutTt_d = nc.dram_tensor("WoutTt", [NVQ, 128, 4, 512], BF,
                              kind="ExternalInput")
    out_d = nc.dram_tensor("out_q", [3, NVQ * 4, 128, 256], F32,
                           kind="ExternalOutput")

    with tile.TileContext(nc) as tc:
        with (
            tc.tile_pool(name="consts", bufs=1) as consts,
            tc.tile_pool(name="big", bufs=1) as big,
            tc.tile_pool(name="state", bufs=2) as state,
            tc.tile_pool(name="work", bufs=3) as work,
            tc.tile_pool(name="wpool", bufs=4) as wpool,
            tc.tile_pool(name="lpool", bufs=3) as lpool,
        ):
            # ---- index load + constants ----
            idx_sb = consts.tile([128, 10, 1], I32)
            nc.gpsimd.dma_start(
                idx_sb[:], idx_d.ap().rearrange("(j p) o -> p j o", p=128))
            identb = consts.tile([128, 128], BF)
            nc.sync.dma_start(identb[:], ident_d.ap())

            WihT_sb = big.tile([128, 4, G4], BF, tag="wih")
            nc.sync.dma_start(WihT_sb[:], WihT_d.ap())
            WinT_sb = big.tile([128, 12, E], BF, tag="win")
            nc.sync.dma_start(WinT_sb[:], WinT_d.ap())
            featT_sb = consts.tile([128, 12, BC], BF)
            nc.sync.dma_start(
                featT_sb[:], featT_d.ap().rearrange("(k p) b -> p k b", p=128))
            WhhT_sb = big.tile([128, 4, G4], BF, tag="whh")
            nc.sync.dma_start(WhhT_sb[:], WhhT_d.ap())
            bias_bc = big.tile([128, G4], F32, tag="biasbc")
            nc.sync.dma_start(
                bias_bc[:],
                bass.AP(tensor=bcomb_d, offset=0, ap=[[0, 128], [1, G4]]))
            bin_sb = consts.tile([128, 4], F32)
            nc.sync.dma_start(
                bin_sb[:], bin_d.ap().rearrange("(k p) -> p k", p=128))


            # ---- embedding gather -> seqT (transposed via PE) ----
            seqT = big.tile([128, 4, NTB], BF, tag="seqT")
            with tc.tile_pool(name="psA", bufs=3, space="PSUM") as psA:
                for j in range(10):
                    gt = work.tile([128, E], BF, tag="gather")
                    nc.gpsimd.indirect_dma_start(
                        out=gt[:], out_offset=None, in_=emb_d.ap(),
                        in_offset=bass.IndirectOffsetOnAxis(
                            ap=idx_sb[:, j, :], axis=0))
                    for e in range(4):
                        pst = psA.tile([128, 128], BF, space="PSUM", tag="tr")
                        nc.tensor.transpose(
                            pst[:], gt[:, e * 128:(e + 1) * 128], identb[:])
                        nc.scalar.copy(
                            seqT[:, e, j * 128:(j + 1) * 128], pst[:])

                # ---- x0T = W_inT.T @ featT + b_in -> seqT[:, :, 0:BC] ----
                for m in range(4):
                    ps = psA.tile([128, BC], F32, space="PSUM", tag="x0")
                    for k in range(12):
                        nc.tensor.matmul(
                            ps[:], lhsT=WinT_sb[:, k, m * 128:(m + 1) * 128],
                            rhs=featT_sb[:, k, :],
                            start=(k == 0), stop=(k == 11))
                    nc.scalar.activation(
                        seqT[:, m, 0:BC], ps[:],
                        mybir.ActivationFunctionType.Identity,
                        bias=bin_sb[:, m:m + 1])

            # ---- LSTM + interleaved xg / vocab GEMM ----
            VWIN = [(0, 16), (16, 32), (32, 40)]
            hid_w = [big.tile([128, 4, t1 - t0, BC], BF, tag=f"hid{w}",
                              name=f"hid{w}")
                     for w, (t0, t1) in enumerate(VWIN)]
            xg_tiles = [big.tile([128, G4], BF, tag=f"xg{mt}", name=f"xg{mt}")
                        for mt in range(10)]

            def hid_of(t):
                for w, (t0, t1) in enumerate(VWIN):
                    if t < t1:
                        return hid_w[w], t - t0
                raise AssertionError

            lstm_ps = tc.tile_pool(name="psGates", bufs=1, space="PSUM")
            htr_ps = tc.tile_pool(name="psHtr", bufs=2, space="PSUM")
            xg_ps = tc.tile_pool(name="psXg", bufs=2, space="PSUM")
            voc_ps = tc.tile_pool(name="psVoc", bufs=2, space="PSUM")
            gpsum = lstm_ps.__enter__()
            tpsum = htr_ps.__enter__()
            xgpsum = xg_ps.__enter__()
            vpsum = [None]

            def emit_xg_mtile(mt):
                for n in range(4):
                    ps = xgpsum.tile([128, 512], F32, space="PSUM", tag="xgps")
                    for k in range(4):
                        nc.tensor.matmul(
                            ps[:],
                            lhsT=seqT[:, k, mt * 128:(mt + 1) * 128],
                            rhs=WihT_sb[:, k, n * 512:(n + 1) * 512],
                            start=(k == 0), stop=(k == 3))
                    nc.vector.tensor_add(
                        xg_tiles[mt][:, n * 512:(n + 1) * 512], ps[:],
                        bias_bc[:, n * 512:(n + 1) * 512])

            vunits = []
            ncopy = [0]

            def emit_vocab_unit(vtq, w):
                t0, t1 = VWIN[w]
                nb = (t1 - t0) * BC
                wt = wpool.tile([128, 4, 512], BF, tag="wout")
                nc.sync.dma_start(wt[:], WoutTt_d.ap()[vtq])
                lsb = lpool.tile([128, 4, 256], F32, tag="lout")
                for pair in range(2):
                    vps = vpsum[0].tile([128, 512], F32, space="PSUM",
                                        tag="vps")
                    for half in range(2):
                        sub = pair * 2 + half
                        hsl = slice(half * 256, half * 256 + nb)
                        for k in range(4):
                            nc.tensor.matmul(
                                vps[:, hsl],
                                lhsT=wt[:, k, sub * 128:(sub + 1) * 128],
                                rhs=hid_w[w][:, k, :, :],
                                start=(k == 0 and half == 0),
                                stop=(k == 3 and half == 1))
                    dst = lsb[:, 2 * pair:2 * pair + 2, 0:256] \
                        .rearrange("p s c -> p (s c)")
                    if pair == 0:
                        nc.scalar.copy(dst, vps[:])
                    else:
                        nc.vector.tensor_copy(dst, vps[:])
                nc.gpsimd.dma_start(
                    out_d.ap()[w, vtq * 4:(vtq + 1) * 4, :, :]
                    .rearrange("s p c -> p s c"),
                    lsb[:])

            emit_xg_mtile(0)
            emit_xg_mtile(1)

            HH = H // 2
            c_prev = None
            for t in range(T):
                if t == 16:
                    xg_ps.__exit__(None, None, None)
                    vpsum[0] = voc_ps.__enter__()
                mt, po = (t * TB) // 128, (t * TB) % 128
                gchunk = {}
                for n in (3, 0, 1, 2):
                    gchunk[n] = gpsum.tile(
                        [BC, 512], F32, space="PSUM", tag=f"gates{n}",
                        name=f"gates{n}")
                sig_i = state.tile([BC, H], F32, tag="sigi")
                sig_f = state.tile([BC, H], F32, tag="sigf")
                sig_o = state.tile([BC, H], F32, tag="sigo")
                g_t = state.tile([BC, H], F32, tag="g")
                act_of = {3: (g_t, mybir.ActivationFunctionType.Tanh),
                          0: (sig_i, mybir.ActivationFunctionType.Sigmoid),
                          1: (sig_f, mybir.ActivationFunctionType.Sigmoid),
                          2: (sig_o, mybir.ActivationFunctionType.Sigmoid)}
                for n in (3, 0, 1, 2):
                    ns = slice(n * 512, (n + 1) * 512)
                    if t > 0:
                        hsrc, trel = hid_of(t - 1)
                        for k in range(4):
                            nc.tensor.matmul(
                                gchunk[n][:],
                                lhsT=hsrc[:, k, trel, :],
                                rhs=WhhT_sb[:, k, ns],
                                start=(k == 0), stop=False)
                    nc.tensor.matmul(
                        gchunk[n][:],
                        lhsT=identb[po:po + BC, po:po + BC],
                        rhs=xg_tiles[mt][po:po + BC, ns],
                        start=(t == 0), stop=True,
                        tile_position=(po, 0))
                    dst, fn = act_of[n]
                    if n == 2:
                        nc.scalar.activation(dst[:, 0:256], gchunk[n][:, 0:256], fn)
                        nc.scalar.activation(dst[:, 256:512], gchunk[n][:, 256:512], fn)
                    else:
                        nc.scalar.activation(dst[:], gchunk[n][:], fn)

                hdst, trel = hid_of(t)
                c_new = [None, None]
                for half in range(2):
                    hs = slice(half * HH, (half + 1) * HH)
                    ig = state.tile([BC, HH], F32, tag=f"ig{half}")
                    nc.vector.tensor_mul(ig[:], sig_i[:, hs], g_t[:, hs])
                    cn = state.tile([BC, HH], F32, tag=f"c{half}")
                    if t == 0:
                        nc.vector.tensor_copy(cn[:], ig[:])
                    else:
                        cf = state.tile([BC, HH], F32, tag=f"cf{half}")
                        nc.vector.tensor_mul(
                            cf[:], sig_f[:, hs], c_prev[half][:])
                        nc.vector.tensor_add(cn[:], cf[:], ig[:])
                    c_new[half] = cn
                    tc_t = state.tile([BC, HH], F32, tag=f"tanhc{half}")
                    nc.scalar.activation(
                        tc_t[:], cn[:], mybir.ActivationFunctionType.Tanh)
                    h_bf = state.tile([BC, HH], BF, tag=f"h{half}")
                    nc.vector.tensor_mul(h_bf[:], sig_o[:, hs], tc_t[:])
                    pst = tpsum.tile([128, 2 * BC], BF, space="PSUM",
                                     tag="htr")
                    for e in range(2):
                        nc.tensor.transpose(
                            pst[:, e * BC:(e + 1) * BC],
                            h_bf[:, e * 128:(e + 1) * 128],
                            identb[0:BC, 0:BC])
                    dstap = hdst[:, 2 * half:2 * half + 2, trel, :]
                    srcap = pst[:].rearrange("p (k b) -> p k b", k=2)
                    if half == 0:
                        nc.scalar.copy(dstap, srcap)
                    else:
                        nc.vector.tensor_copy(dstap, srcap)
                c_prev = c_new

                # interleaved filler work
                if t < 16 and t % 2 == 0 and t // 2 + 2 < 10:
                    emit_xg_mtile(t // 2 + 2)
                for w, (t0, t1) in enumerate(VWIN):
                    if t == t1 - 1:
                        vunits.extend((vtq, w) for vtq in range(NVQ))
                if t >= 16:
                    for _ in range(2 if t % 2 else 1):
                        if vunits:
                            emit_vocab_unit(*vunits.pop(0))

            # vocab tail
            while vunits:
                emit_vocab_unit(*vunits.pop(0))

            voc_ps.__exit__(None, None, None)
            htr_ps.__exit__(None, None, None)
            lstm_ps.__exit__(None, None, None)

    nc.compile()
    _CACHE["nc"] = nc
    return nc


def kernel(features, seqs, lengths, W_in, b_in, emb, W_ih, W_hh, b_ih, b_hh,
           W_out, b_out):
    f32 = lambda x: np.asarray(x, dtype=np.float32)
    bf = lambda x: np.ascontiguousarray(f32(x)).astype(bfnp)
    features, seqs = f32(features), np.asarray(seqs).astype(np.int64)
    # gate order [i, f, o, g]
    perm = np.concatenate([np.arange(0, 2 * H), np.arange(3 * H, 4 * H),
                           np.arange(2 * H, 3 * H)])
    WinT = np.ascontiguousarray(
        bf(f32(W_in).T).reshape(12, 128, E).transpose(1, 0, 2))
    WihT = np.ascontiguousarray(
        bf(f32(W_ih).T)[:, perm].reshape(4, 128, G4).transpose(1, 0, 2))
    WhhT = np.ascontiguousarray(
        bf(f32(W_hh).T)[:, perm].reshape(4, 128, G4).transpose(1, 0, 2))
    bcomb = np.ascontiguousarray((f32(b_ih) + f32(b_hh))[perm])
    emb_b = bf(emb)
    WoutT = np.zeros((H, VP), dtype=bfnp)
    WoutT[:, :V] = bf(f32(W_out).T)
    # quad-tiled layout [vtq, p, k, v4]: element = WoutT[k*128+p, vtq*512+v4]
    WoutTt = np.ascontiguousarray(
        WoutT.reshape(4, 128, NVQ, 512).transpose(2, 1, 0, 3))
    ident_np = np.eye(128, dtype=bfnp)
    binp = f32(b_in)

    nc = _build()
    in_maps = []
    for c in range(NCORES):
        bs = slice(c * BC, (c + 1) * BC)
        featT = bf(features[bs].T)             # [F, BC]
        idx = np.zeros((T, TB), np.int64)
        idx[1:, :BC] = seqs[bs].T              # t-major, t=0 block dummy
        in_maps.append({
            "featT": featT,
            "idx": idx.reshape(NTB, 1).astype(np.int32),
            "embt": emb_b,
            "WinT": WinT, "WihT": WihT, "WhhT": WhhT,
            "bcomb": bcomb, "bin": binp, "ident": ident_np,
            "WoutTt": WoutTt,
        })
    _CACHE["last_in_maps"] = in_maps
    res = run_bass_kernel_spmd(nc, in_maps, list(range(NCORES)))
    out = np.empty((B, T, V), np.float32)
    wlens = [256, 256, 128]
    for c in range(NCORES):
        oq = res.results[c]["out_q"]           # [3, 80, 128, 256]
        parts = [oq[w].reshape(VP, 256)[:V, :wlens[w]] for w in range(3)]
        lt = np.concatenate(parts, axis=1)     # [V, 640]
        out[c * BC:(c + 1) * BC] = (
            lt.reshape(V, T, BC).transpose(2, 1, 0))
    bo = f32(b_out)
    if np.any(bo):
        out += bo
    return out


# revision 30
# speedup vs baseline: 1.1400x; 1.0927x over previous
"""Trainium2 Bass kernel for nn_Caption (LSTM caption decoder).

Distribution: pure data-parallel over batch (128 -> 8 cores x 16), no
collectives. Per core: x0 projection GEMM, embedding gather (device),
input-gate GEMM, 40-step LSTM recurrence, vocab GEMM [640,512]@[512,10000].

Layout strategy: all GEMM operands bf16 (fp32 PSUM accumulation); weights
host-transposed so the contraction dim lands on partitions; outputs
produced in T-layout (feature on partitions) so biases fuse into ACT
copies as per-partition bias. LSTM runs B-layout (batch on partitions)
with per-step h transposed via PE into hiddensT, which is consumed
directly by the vocab GEMM. xg is injected into the gates PSUM via
identity matmuls (t-blocks padded to 32 partitions for alignment).
"""
import sys

sys.path.insert(0, "/opt/trn_rl_repo")

import numpy as np
import ml_dtypes

import concourse.bass as bass
import concourse.tile as tile
from concourse import bacc, mybir
from concourse.bass_utils import run_bass_kernel_spmd
from concourse.masks import make_identity

BF = mybir.dt.bfloat16
F32 = mybir.dt.float32
I32 = mybir.dt.int32
bfnp = ml_dtypes.bfloat16

B, F, E, H, V, T = 128, 1536, 512, 512, 10000, 40
NCORES = 8
BC = B // NCORES          # 16 batch rows per core
TB = 32                   # padded t-block width (partition alignment)
NTB = T * TB              # 1280 padded (t,b) columns
NB = T * BC               # 640 real (t,b) columns
G4 = 4 * H                # 2048 gate dims, order [i, f, o, g]
VP = 10240               # padded vocab (80 tiles of 128, 20 quads)
NVT = VP // 128           # 80 vocab tiles
NVQ = NVT // 4            # 20 vocab quads

_CACHE = {}


def _build():
    if "nc" in _CACHE:
        return _CACHE["nc"]
    nc = bacc.Bacc("TRN2", target_bir_lowering=False, debug=False,
                   num_devices=NCORES)

    featT_d = nc.dram_tensor("featT", [F, BC], BF, kind="ExternalInput")
    idx_d = nc.dram_tensor("idx", [NTB, 1], I32, kind="ExternalInput")
    emb_d = nc.dram_tensor("embt", [V, E], BF, kind="ExternalInput")
    WinT_d = nc.dram_tensor("WinT", [128, 12, E], BF, kind="ExternalInput")
    WihT_d = nc.dram_tensor("WihT", [128, 4, G4], BF, kind="ExternalInput")
    WhhT_d = nc.dram_tensor("WhhT", [128, 4, G4], BF, kind="ExternalInput")
    bcomb_d = nc.dram_tensor("bcomb", [G4], F32, kind="ExternalInput")
    bin_d = nc.dram_tensor("bin", [E], F32, kind="ExternalInput")
    ident_d = nc.dram_tensor("ident", [128, 128], BF, kind="ExternalInput")
    WoutTt_d = nc.dram_tensor("WoutTt", [NVQ, 128, 4, 512], BF,
                              kind="ExternalInput")
    out_d = nc.dram_tensor("out_q", [3, NVQ * 4, 128, 256], F32,
                           kind="ExternalOutput")

    with tile.TileContext(nc) as tc:
        with (
            tc.tile_pool(name="consts", bufs=1) as consts,
            tc.tile_pool(name="big", bufs=1) as big,
            tc.tile_pool(name="state", bufs=2) as state,
            tc.tile_pool(name="work", bufs=3) as work,
            tc.tile_pool(name="wpool", bufs=4) as wpool,
            tc.tile_pool(name="lpool", bufs=3) as lpool,
        ):
            # ---- index load + constants ----
            idx_sb = consts.tile([128, 10, 1], I32)
            nc.sync.dma_start(
                idx_sb[:], idx_d.ap().rearrange("(j p) o -> p j o", p=128))
            identb = consts.tile([128, 128], BF)
            nc.sync.dma_start(identb[:], ident_d.ap())

            WihT_sb = big.tile([128, 4, G4], BF, tag="wih")
            nc.sync.dma_start(WihT_sb[:], WihT_d.ap())
            WinT_sb = big.tile([128, 12, E], BF, tag="win")
            nc.sync.dma_start(WinT_sb[:], WinT_d.ap())
            featT_sb = consts.tile([128, 12, BC], BF)
            nc.sync.dma_start(
                featT_sb[:], featT_d.ap().rearrange("(k p) b -> p k b", p=128))
            WhhT_sb = big.tile([128, 4, G4], BF, tag="whh")
            nc.sync.dma_start(WhhT_sb[:], WhhT_d.ap())
            bias_bc = big.tile([128, G4], F32, tag="biasbc")
            nc.sync.dma_start(
                bias_bc[:],
                bass.AP(tensor=bcomb_d, offset=0, ap=[[0, 128], [1, G4]]))
            bin_sb = consts.tile([128, 4], F32)
            nc.sync.dma_start(
                bin_sb[:], bin_d.ap().rearrange("(k p) -> p k", p=128))


            # ---- embedding gather -> seqT (transposed via PE) ----
            seqT = big.tile([128, 4, NTB], BF, tag="seqT")
            with tc.tile_pool(name="psA", bufs=3, space="PSUM") as psA:
                for j in range(10):
                    gt = work.tile([128, E], BF, tag="gather")
                    nc.gpsimd.indirect_dma_start(
                        out=gt[:], out_offset=None, in_=emb_d.ap(),
                        in_offset=bass.IndirectOffsetOnAxis(
                            ap=idx_sb[:, j, :], axis=0))
                    for e in range(4):
                        pst = psA.tile([128, 128], BF, space="PSUM", tag="tr")
                        nc.tensor.transpose(
                            pst[:], gt[:, e * 128:(e + 1) * 128], identb[:])
                        nc.scalar.copy(
                            seqT[:, e, j * 128:(j + 1) * 128], pst[:])

                # ---- x0T = W_inT.T @ featT + b_in -> seqT[:, :, 0:BC] ----
                for m in range(4):
                    ps = psA.tile([128, BC], F32, space="PSUM", tag="x0")
                    for k in range(12):
                        nc.tensor.matmul(
                            ps[:], lhsT=WinT_sb[:, k, m * 128:(m + 1) * 128],
                            rhs=featT_sb[:, k, :],
                            start=(k == 0), stop=(k == 11))
                    nc.scalar.activation(
                        seqT[:, m, 0:BC], ps[:],
                        mybir.ActivationFunctionType.Identity,
                        bias=bin_sb[:, m:m + 1])

            # ---- LSTM + interleaved xg / vocab GEMM ----
            VWIN = [(0, 16), (16, 32), (32, 40)]
            hid_w = [big.tile([128, 4, t1 - t0, BC], BF, tag=f"hid{w}",
                              name=f"hid{w}")
                     for w, (t0, t1) in enumerate(VWIN)]
            xg_tiles = [big.tile([128, G4], BF, tag=f"xg{mt}", name=f"xg{mt}")
                        for mt in range(10)]

            def hid_of(t):
                for w, (t0, t1) in enumerate(VWIN):
                    if t < t1:
                        return hid_w[w], t - t0
                raise AssertionError

            lstm_ps = tc.tile_pool(name="psGates", bufs=1, space="PSUM")
            htr_ps = tc.tile_pool(name="psHtr", bufs=2, space="PSUM")
            xg_ps = tc.tile_pool(name="psXg", bufs=2, space="PSUM")
            voc_ps = tc.tile_pool(name="psVoc", bufs=2, space="PSUM")
            gpsum = lstm_ps.__enter__()
            tpsum = htr_ps.__enter__()
            xgpsum = xg_ps.__enter__()
            vpsum = [None]

            def emit_xg_mtile(mt):
                for n in range(4):
                    ps = xgpsum.tile([128, 512], F32, space="PSUM", tag="xgps")
                    for k in range(4):
                        nc.tensor.matmul(
                            ps[:],
                            lhsT=seqT[:, k, mt * 128:(mt + 1) * 128],
                            rhs=WihT_sb[:, k, n * 512:(n + 1) * 512],
                            start=(k == 0), stop=(k == 3))
                    nc.vector.tensor_add(
                        xg_tiles[mt][:, n * 512:(n + 1) * 512], ps[:],
                        bias_bc[:, n * 512:(n + 1) * 512])

            vunits = []
            ncopy = [0]

            def emit_vocab_unit(vtq, w):
                t0, t1 = VWIN[w]
                nb = (t1 - t0) * BC
                wt = wpool.tile([128, 4, 512], BF, tag="wout")
                nc.sync.dma_start(wt[:], WoutTt_d.ap()[vtq])
                lsb = lpool.tile([128, 4, 256], F32, tag="lout")
                for pair in range(2):
                    vps = vpsum[0].tile([128, 512], F32, space="PSUM",
                                        tag="vps")
                    for half in range(2):
                        sub = pair * 2 + half
                        hsl = slice(half * 256, half * 256 + nb)
                        for k in range(4):
                            nc.tensor.matmul(
                                vps[:, hsl],
                                lhsT=wt[:, k, sub * 128:(sub + 1) * 128],
                                rhs=hid_w[w][:, k, :, :],
                                start=(k == 0 and half == 0),
                                stop=(k == 3 and half == 1))
                    dst = lsb[:, 2 * pair:2 * pair + 2, 0:256] \
                        .rearrange("p s c -> p (s c)")
                    if pair == 0:
                        nc.scalar.copy(dst, vps[:])
                    else:
                        nc.vector.tensor_copy(dst, vps[:])
                nc.gpsimd.dma_start(
                    out_d.ap()[w, vtq * 4:(vtq + 1) * 4, :, :]
                    .rearrange("s p c -> p s c"),
                    lsb[:])

            emit_xg_mtile(0)
            emit_xg_mtile(1)

            HH = H // 2
            c_prev = None
            for t in range(T):
                if t == 16:
                    xg_ps.__exit__(None, None, None)
                    vpsum[0] = voc_ps.__enter__()
                mt, po = (t * TB) // 128, (t * TB) % 128
                gchunk = {}
                for n in (3, 0, 1, 2):
                    gchunk[n] = gpsum.tile(
                        [BC, 512], F32, space="PSUM", tag=f"gates{n}",
                        name=f"gates{n}")
                sig_i = state.tile([BC, H], F32, tag="sigi")
                sig_f = state.tile([BC, H], F32, tag="sigf")
                sig_o = state.tile([BC, H], F32, tag="sigo")
                g_t = state.tile([BC, H], F32, tag="g")
                act_of = {3: (g_t, mybir.ActivationFunctionType.Tanh),
                          0: (sig_i, mybir.ActivationFunctionType.Sigmoid),
                          1: (sig_f, mybir.ActivationFunctionType.Sigmoid),
                          2: (sig_o, mybir.ActivationFunctionType.Sigmoid)}
                for n in (3, 0, 1, 2):
                    ns = slice(n * 512, (n + 1) * 512)
                    nc.tensor.matmul(
                        gchunk[n][:],
                        lhsT=identb[po:po + BC, po:po + BC],
                        rhs=xg_tiles[mt][po:po + BC, ns],
                        start=True, stop=(t == 0),
                        tile_position=(po, 0))
                    if t > 0:
                        hsrc, trel = hid_of(t - 1)
                        for k in range(4):
                            nc.tensor.matmul(
                                gchunk[n][:],
                                lhsT=hsrc[:, k, trel, :],
                                rhs=WhhT_sb[:, k, ns],
                                start=False, stop=(k == 3))
                    dst, fn = act_of[n]
                    if n == 2:
                        nc.scalar.activation(dst[:, 0:256], gchunk[n][:, 0:256], fn)
                        nc.scalar.activation(dst[:, 256:512], gchunk[n][:, 256:512], fn)
                    else:
                        nc.scalar.activation(dst[:], gchunk[n][:], fn)

                hdst, trel = hid_of(t)
                c_new = [None, None]
                for half in range(2):
                    hs = slice(half * HH, (half + 1) * HH)
                    ig = state.tile([BC, HH], F32, tag=f"ig{half}")
                    nc.vector.tensor_mul(ig[:], sig_i[:, hs], g_t[:, hs])
                    cn = state.tile([BC, HH], F32, tag=f"c{half}")
                    if t == 0:
                        nc.vector.tensor_copy(cn[:], ig[:])
                    else:
                        cf = state.tile([BC, HH], F32, tag=f"cf{half}")
                        nc.vector.tensor_mul(
                            cf[:], sig_f[:, hs], c_prev[half][:])
                        nc.vector.tensor_add(cn[:], cf[:], ig[:])
                    c_new[half] = cn
                    tc_t = state.tile([BC, HH], F32, tag=f"tanhc{half}")
                    nc.scalar.activation(
                        tc_t[:], cn[:], mybir.ActivationFunctionType.Tanh)
                    h_bf = state.tile([BC, HH], BF, tag=f"h{half}")
                    nc.vector.tensor_mul(h_bf[:], sig_o[:, hs], tc_t[:])
                    pst = tpsum.tile([128, 2 * BC], BF, space="PSUM",
                                     tag="htr")
                    for e in range(2):
                        nc.tensor.transpose(
                            pst[:, e * BC:(e + 1) * BC],
                            h_bf[:, e * 128:(e + 1) * 128],
                            identb[0:BC, 0:BC])
                    dstap = hdst[:, 2 * half:2 * half + 2, trel, :]
                    srcap = pst[:].rearrange("p (k b) -> p k b", k=2)
                    if half == 0:
                        nc.scalar.copy(dstap, srcap)
                    else:
                        nc.vector.tensor_copy(dstap, srcap)
                c_prev = c_new

                # interleaved filler work
                if t < 16 and t % 2 == 0 and t // 2 + 2 < 10:
                    emit_xg_mtile(t // 2 + 2)
                for w, (t0, t1) in enumerate(VWIN):
                    if t == t1 - 1:
                        vunits.extend((vtq, w) for vtq in range(NVQ))
                if t >= 16:
                    if vunits:
                        emit_vocab_unit(*vunits.pop(0))

            voc_ps.__exit__(None, None, None)
            htr_ps.__exit__(None, None, None)
            lstm_ps.__exit__(None, None, None)

            # vocab tail with wide PSUM pool
            with tc.tile_pool(name="psVoc2", bufs=6, space="PSUM") as vp2:
                vpsum[0] = vp2
                while vunits:
                    emit_vocab_unit(*vunits.pop(0))

    nc.compile()
    _CACHE["nc"] = nc
    return nc


def kernel(features, seqs, lengths, W_in, b_in, emb, W_ih, W_hh, b_ih, b_hh,
           W_out, b_out):
    f32 = lambda x: np.asarray(x, dtype=np.float32)
    bf = lambda x: np.ascontiguousarray(f32(x)).astype(bfnp)
    features, seqs = f32(features), np.asarray(seqs).astype(np.int64)
    # gate order [i, f, o, g]
    perm = np.concatenate([np.arange(0, 2 * H), np.arange(3 * H, 4 * H),
                           np.arange(2 * H, 3 * H)])
    WinT = np.ascontiguousarray(
        bf(f32(W_in).T).reshape(12, 128, E).transpose(1, 0, 2))
    WihT = np.ascontiguousarray(
        bf(f32(W_ih).T)[:, perm].reshape(4, 128, G4).transpose(1, 0, 2))
    WhhT = np.ascontiguousarray(
        bf(f32(W_hh).T)[:, perm].reshape(4, 128, G4).transpose(1, 0, 2))
    bcomb = np.ascontiguousarray((f32(b_ih) + f32(b_hh))[perm])
    emb_b = bf(emb)
    WoutT = np.zeros((H, VP), dtype=bfnp)
    WoutT[:, :V] = bf(f32(W_out).T)
    # quad-tiled layout [vtq, p, k, v4]: element = WoutT[k*128+p, vtq*512+v4]
    WoutTt = np.ascontiguousarray(
        WoutT.reshape(4, 128, NVQ, 512).transpose(2, 1, 0, 3))
    ident_np = np.eye(128, dtype=bfnp)
    binp = f32(b_in)

    nc = _build()
    in_maps = []
    for c in range(NCORES):
        bs = slice(c * BC, (c + 1) * BC)
        featT = bf(features[bs].T)             # [F, BC]
        idx = np.zeros((T, TB), np.int64)
        idx[1:, :BC] = seqs[bs].T              # t-major, t=0 block dummy
        in_maps.append({
            "featT": featT,
            "idx": idx.reshape(NTB, 1).astype(np.int32),
            "embt": emb_b,
            "WinT": WinT, "WihT": WihT, "WhhT": WhhT,
            "bcomb": bcomb, "bin": binp, "ident": ident_np,
            "WoutTt": WoutTt,
        })
    _CACHE["last_in_maps"] = in_maps
    res = run_bass_kernel_spmd(nc, in_maps, list(range(NCORES)))
    out = np.empty((B, T, V), np.float32)
    wlens = [256, 256, 128]
    for c in range(NCORES):
        oq = res.results[c]["out_q"]           # [3, 80, 128, 256]
        parts = [oq[w].reshape(VP, 256)[:V, :wlens[w]] for w in range(3)]
        lt = np.concatenate(parts, axis=1)     # [V, 640]
        out[c * BC:(c + 1) * BC] = (
            lt.reshape(V, T, BC).transpose(2, 1, 0))
    bo = f32(b_out)
    if np.any(bo):
        out += bo
    return out
